# revision 12
# baseline (speedup 1.0000x reference)
"""Trainium2 Bass kernel for CompanySpecificHeads (MoE-style routed MLP heads).

Semantics (matching the reference):
    out[b] = gelu(z[b] @ W1[cid[b]] + b1[cid[b]]) @ W2[cid[b]] + b2[cid[b]]

Strategy: expert-parallel across 8 NeuronCores. Companies are sharded
8-per-core; tokens are routed (gathered by company) to their company's core
on the host, padded to a fixed per-company capacity, and each core runs a
grouped GEMM -> gelu -> dot pipeline over its 8 companies:

  Layer 1 (per company c, h on partitions):
      psum[h, t] = sum_d W1[c][d, h] * zT[c][d, t]      (PE, fp16 operands)
  Gelu: ACT engine, PSUM -> SBUF (fp16 out), with b1 folded in via the
      per-partition bias operand of the activation instruction.
  Layer 2: psum2[1, t] += W2[c][hj]^T @ gelu_h[hj, t]   (8 K=128 matmuls)

Host does the unshard/scatter back to [B, 1] and adds b2 (exact, fp32).

Schedule notes (from trace analysis of the previous version):
  - The w1 stream is the critical path (8MB/core at ~341GB/s ~= 24.6us), so
    its first DMA trigger is the first instruction on the sync ring after
    the framework preamble; small loads (b1/w2/z) ride the scalar ring.
  - PE warmup matmuls depend only on a vector-engine memset so they start
    right after the preamble; the HAM clock gate then un-throttles the PE
    (1.2 -> 2.4 GHz) before the first real matmul instead of 12us in.
  - Layer-2 matmuls for group (c,g) are emitted after layer-1 of the next
    group so the PE never stalls waiting on the ACT engine's gelu.
  - Outputs are staged in SBUF and stored with HWDGE (sync ring) in two
    chunks: companies 0-6 overlap company 7's compute; the final store
    carries only company 7 (sub-us tail).
"""

import numpy as np

B, C, D, H = 4096, 64, 512, 1024
NCORES = 8
CPC = C // NCORES  # companies per core
KC = D // 128      # contraction chunks of 128
HC = H // 128      # h chunks of 128

_COMPILED = {}


def _build(TW, NTT, dtype_name):
    """Build the Bass/Tile program for per-company token capacity NTT*TW."""
    import concourse.bass as bass
    import concourse.bacc as bacc
    import concourse.mybir as mybir
    from concourse.tile import TileContext
    from contextlib import ExitStack

    f32 = mybir.dt.float32
    dt_op = getattr(mybir.dt, dtype_name)

    nc = bacc.Bacc(None, target_bir_lowering=False)

    # zt is stored partition-major so one DMA moves it with large packets.
    zt_d = nc.dram_tensor("zt", [128, CPC, NTT, KC, TW], dt_op, kind="ExternalInput")
    # w1 stored as [c][p][g][k][h-half]: a whole company loads linearly
    # with 8KB contiguous per partition (full-rate packets).
    w1_d = nc.dram_tensor(
        "w1", [CPC, 128, 2, KC, H // 2], dt_op, kind="ExternalInput"
    )
    # b1 as columns: b1c[m, (c*2+g)*KC + j] = b1[c][512g+128j+m], fp32.
    b1_d = nc.dram_tensor("b1c", [128, CPC * 2 * KC], f32, kind="ExternalInput")
    w2_d = nc.dram_tensor("w2h", [128, CPC * HC], dt_op, kind="ExternalInput")
    out_d = nc.dram_tensor("out", [1, CPC * NTT * TW], f32, kind="ExternalOutput")

    gelu = mybir.ActivationFunctionType.Gelu

    with TileContext(nc) as tc, ExitStack() as ctx:
        const = ctx.enter_context(tc.tile_pool(name="const", bufs=1))

        # Per-company weights on the SP HWDGE ring, one company per DMA,
        # issued before anything else so the stream starts as early as
        # possible. The ring drains FIFO at full bandwidth and compute
        # pipelines behind the weight stream.
        w1p = ctx.enter_context(tc.tile_pool(name="w1p", bufs=1))
        w1ts = []
        for c in range(CPC):
            w1t = w1p.tile([128, 2, KC, H // 2], dt_op, name=f"w1_{c}")
            nc.sync.dma_start(out=w1t[:], in_=w1_d[c])
            w1ts.append(w1t)

        # Small constants + routed tokens on the ACT HWDGE ring: its
        # dispatch overlaps the SP ring's w1 dispatches. Keep the total
        # HWDGE DMA count low — only ~8 trigger lanes exist, and a spare
        # trigger can only fire when an earlier DMA on its lane finishes.
        # The small scalar loads all complete early, so the later w1
        # triggers that pace on them are never the bottleneck.
        zall = const.tile([128, CPC, NTT, KC, TW], dt_op)
        b1t = const.tile([128, CPC * 2 * KC], f32)
        w2t = const.tile([128, CPC * HC], dt_op)
        nc.scalar.dma_start(out=zall[:, 0:1], in_=zt_d[:, 0:1])
        nc.scalar.dma_start(out=b1t[:], in_=b1_d[:])
        nc.scalar.dma_start(out=w2t[:], in_=w2_d[:])
        nc.scalar.dma_start(out=zall[:, 1:], in_=zt_d[:, 1:])

        # Staged per-company outputs; two HWDGE stores at the end.
        oall = const.tile([1, CPC * NTT * TW], f32)

        hp = ctx.enter_context(tc.tile_pool(name="hp", bufs=4))
        pp = ctx.enter_context(tc.tile_pool(name="pp", bufs=4, space="PSUM"))
        opp = ctx.enter_context(tc.tile_pool(name="opp", bufs=2, space="PSUM"))

        # PE warmup: dependency-free matmuls on scratch data so the HAM
        # clock gate un-throttles the PE (1.2 -> 2.4 GHz takes ~3.4us of
        # sustained activity) while the first w1 DMA streams in. The
        # scratch memset runs on the otherwise-idle vector engine so the
        # warmup starts right after the framework preamble.
        wsc = const.tile([128, 384], dt_op)
        nc.vector.memset(wsc[:], 0.0)
        wps = ctx.enter_context(tc.tile_pool(name="wps", bufs=1, space="PSUM"))
        wp = wps.tile([128, 384], f32)
        for _ in range(10):
            nc.tensor.matmul(wp[:], wsc[:, :128], wsc[:], start=True, stop=True)

        # Software-pipelined job list: layer-2 of job i is emitted after
        # layer-1 of job i+1, so the PE keeps streaming layer-1 matmuls
        # while the ACT engine computes job i's gelu.
        jobs = [(c, tt, g) for c in range(CPC) for tt in range(NTT) for g in range(2)]
        pending = []  # (c, tt, g, ps_or_ht state)
        osums = {}

        def emit_l1(c, tt, g):
            ps = pp.tile([128, KC * TW], f32)
            for j in range(KC):
                for k in range(KC):
                    nc.tensor.matmul(
                        ps[:, j * TW:(j + 1) * TW],
                        w1ts[c][:, g, k, 128 * j:128 * (j + 1)],
                        zall[:, c, tt, k, :],
                        start=(k == 0),
                        stop=(k == KC - 1),
                    )
            # Bias on the (otherwise idle) vector engine: ONE in-place
            # broadcast add into PSUM per group. A stride-0 AP repeats
            # each b1 value across the token axis, so the whole group is
            # a single DVE instruction — per-instruction fixed cost
            # (~0.4us) on DVE/ACT is what paced the previous versions.
            idx = (c * 2 + g) * KC
            bias_b = b1t[:, idx:idx + KC].unsqueeze(2).broadcast_to([128, KC, TW])
            psv = ps[:].rearrange("p (j t) -> p j t", j=KC)
            nc.vector.tensor_add(psv, psv, bias_b)
            ht = hp.tile([128, KC * TW], dt_op)
            nc.scalar.activation(ht[:], ps[:], gelu)
            return ht

        def emit_l2(c, tt, g, ht):
            if g == 0:
                osums[(c, tt)] = opp.tile([1, TW], f32, name="osum")
            osum = osums[(c, tt)]
            for j in range(KC):
                jj = KC * g + j
                nc.tensor.matmul(
                    osum[:],
                    w2t[:, HC * c + jj:HC * c + jj + 1],
                    ht[:, j * TW:(j + 1) * TW],
                    start=(jj == 0),
                    stop=(jj == HC - 1),
                )
            if g == 1:
                off = (c * NTT + tt) * TW
                nc.vector.tensor_copy(oall[:, off:off + TW], osum[:])
                del osums[(c, tt)]

        for job in jobs:
            ht = emit_l1(*job)
            if pending:
                emit_l2(*pending.pop(0))
            pending.append((*job, ht))
        while pending:
            emit_l2(*pending.pop(0))

        # Stores on the sync ring (drained long before): companies 0-6
        # fire while company 7 computes; the final store is tiny.
        osplit = (CPC - 1) * NTT * TW
        nc.sync.dma_start(out=out_d[:, :osplit], in_=oall[:, :osplit])
        nc.sync.dma_start(out=out_d[:, osplit:], in_=oall[:, osplit:])

    nc.finalize()
    return nc


def _get_compiled(TW, NTT, dtype_name):
    key = (TW, NTT, dtype_name)
    if key not in _COMPILED:
        _COMPILED[key] = _build(TW, NTT, dtype_name)
    return _COMPILED[key]


def kernel(z, company_id, W1, b1, W2, b2):
    from concourse.bass_utils import run_bass_kernel_spmd

    z = np.asarray(z, dtype=np.float32)
    cid = np.asarray(company_id).astype(np.int64).ravel()
    W1 = np.asarray(W1, dtype=np.float32)
    b1 = np.asarray(b1, dtype=np.float32)
    W2 = np.asarray(W2, dtype=np.float32)
    b2 = np.asarray(b2, dtype=np.float32)
    O = W2.shape[2]

    np_op = np.float16
    dtype_name = "float16"

    idx_by_company = [np.nonzero(cid == gc)[0] for gc in range(C)]
    max_cnt = max((len(ix) for ix in idx_by_company), default=1)
    max_cnt = max(max_cnt, 1)
    if max_cnt <= 128:
        NTT = 1
        TW = ((max_cnt + 15) // 16) * 16
    else:
        NTT = (max_cnt + 127) // 128
        TW = 128
    CAP = NTT * TW

    nc = _get_compiled(TW, NTT, dtype_name)

    in_maps = []
    for core in range(NCORES):
        # zt[p, c, tt, k, t] = z[token, 128k+p]  (partition-major)
        zt = np.zeros((128, CPC, NTT, KC, TW), dtype=np_op)
        for ci in range(CPC):
            gc = core * CPC + ci
            ix = idx_by_company[gc]
            if len(ix) == 0:
                continue
            zpad = np.zeros((CAP, D), dtype=np_op)
            zpad[: len(ix)] = z[ix].astype(np_op)
            zt[:, ci] = zpad.reshape(NTT, TW, KC, 128).transpose(3, 0, 2, 1)
        # w1[c, p, g, k, hh] = W1[gc, 128k+p, 512g+hh]
        w1 = (
            W1[core * CPC:(core + 1) * CPC]
            .reshape(CPC, KC, 128, 2, H // 2)
            .transpose(0, 2, 3, 1, 4)
            .astype(np_op)
        )
        # b1c[m, (c*2+g)*KC + j] = b1[gc, 512g+128j+m]  (fp32 columns)
        b1c = (
            b1[core * CPC:(core + 1) * CPC]
            .reshape(CPC, 2, KC, 128)
            .transpose(3, 0, 1, 2)
            .reshape(128, CPC * 2 * KC)
            .astype(np.float32)
        )
        # w2h[p, HC*c + j] = W2[gc, 128j+p, 0]
        w2h = (
            W2[core * CPC:(core + 1) * CPC, :, 0]
            .reshape(CPC, HC, 128)
            .transpose(2, 0, 1)
            .reshape(128, CPC * HC)
            .astype(np_op)
        )
        in_maps.append(
            {
                "zt": np.ascontiguousarray(zt),
                "w1": np.ascontiguousarray(w1),
                "b1c": np.ascontiguousarray(b1c),
                "w2h": np.ascontiguousarray(w2h),
            }
        )

    res = run_bass_kernel_spmd(nc, in_maps, list(range(NCORES)))

    out = np.zeros((B, O), dtype=np.float32)
    for core in range(NCORES):
        core_out = res.results[core]["out"].reshape(CPC, NTT * TW)
        for ci in range(CPC):
            gc = core * CPC + ci
            ix = idx_by_company[gc]
            if len(ix) == 0:
                continue
            out[ix, 0] = core_out[ci, : len(ix)] + b2[gc, 0]
    return out


# revision 13
# speedup vs baseline: 1.0397x; 1.0397x over previous
"""Trainium2 Bass kernel for CompanySpecificHeads (MoE-style routed MLP heads).

Semantics (matching the reference):
    out[b] = gelu(z[b] @ W1[cid[b]] + b1[cid[b]]) @ W2[cid[b]] + b2[cid[b]]

Strategy: expert-parallel across 8 NeuronCores. Companies are sharded
8-per-core; tokens are routed (gathered by company) to their company's core
on the host, padded to a fixed per-company capacity, and each core runs a
grouped GEMM -> gelu -> dot pipeline over its 8 companies:

  Layer 1 (per company c, h on partitions):
      psum[h, t] = sum_d W1[c][d, h] * zT[c][d, t]      (PE, fp16 operands)
  Gelu: ACT engine, PSUM -> SBUF (fp16 out), with b1 folded in via the
      per-partition bias operand of the activation instruction.
  Layer 2: psum2[1, t] += W2[c][hj]^T @ gelu_h[hj, t]   (8 K=128 matmuls)

Host does the unshard/scatter back to [B, 1] and adds b2 (exact, fp32).

Schedule notes (from trace analysis of the previous version):
  - The w1 stream is the critical path (8MB/core at ~341GB/s ~= 24.6us), so
    its first DMA trigger is the first instruction on the sync ring after
    the framework preamble; small loads (b1/w2/z) ride the scalar ring.
  - PE warmup matmuls depend only on a vector-engine memset so they start
    right after the preamble; the HAM clock gate then un-throttles the PE
    (1.2 -> 2.4 GHz) before the first real matmul instead of 12us in.
  - Layer-2 matmuls for group (c,g) are emitted after layer-1 of the next
    group so the PE never stalls waiting on the ACT engine's gelu.
  - Outputs are staged in SBUF and stored with HWDGE (sync ring) in two
    chunks: companies 0-6 overlap company 7's compute; the final store
    carries only company 7 (sub-us tail).
"""

import numpy as np

B, C, D, H = 4096, 64, 512, 1024
NCORES = 8
CPC = C // NCORES  # companies per core
KC = D // 128      # contraction chunks of 128
HC = H // 128      # h chunks of 128

_COMPILED = {}


def _build(TW, NTT, dtype_name):
    """Build the Bass/Tile program for per-company token capacity NTT*TW."""
    import concourse.bass as bass
    import concourse.bacc as bacc
    import concourse.mybir as mybir
    from concourse.tile import TileContext
    from contextlib import ExitStack

    f32 = mybir.dt.float32
    dt_op = getattr(mybir.dt, dtype_name)

    nc = bacc.Bacc(None, target_bir_lowering=False)

    # zt is stored partition-major so one DMA moves it with large packets.
    zt_d = nc.dram_tensor("zt", [128, CPC, NTT, KC, TW], dt_op, kind="ExternalInput")
    # w1 stored as [c][p][g][k][h-half]: a whole company loads linearly
    # with 8KB contiguous per partition (full-rate packets).
    w1_d = nc.dram_tensor(
        "w1", [CPC, 128, 2, KC, H // 2], dt_op, kind="ExternalInput"
    )
    # b1 as columns: b1c[m, (c*2+g)*KC + j] = b1[c][512g+128j+m], fp32.
    b1_d = nc.dram_tensor("b1c", [128, CPC * 2 * KC], f32, kind="ExternalInput")
    w2_d = nc.dram_tensor("w2h", [128, CPC * HC], dt_op, kind="ExternalInput")
    out_d = nc.dram_tensor("out", [1, CPC * NTT * TW], f32, kind="ExternalOutput")

    gelu = mybir.ActivationFunctionType.Gelu

    with TileContext(nc) as tc, ExitStack() as ctx:
        const = ctx.enter_context(tc.tile_pool(name="const", bufs=1))

        # Per-company weights on the SP HWDGE ring, one company per DMA,
        # issued before anything else so the stream starts as early as
        # possible. The ring drains FIFO at full bandwidth and compute
        # pipelines behind the weight stream.
        # All BULK data rides the sync (SP HWDGE) ring in need-order: the
        # scalar ring only sustains ~85GB/s when competing with the sync
        # ring, so a bulk transfer there starves the pipeline. The sync
        # ring is FIFO, which also gives deterministic arrival order.
        # Tokens for companies 2-7 slot in after w1[1]; they are needed
        # from company 2 onward (~6us later). Small early loads (tokens
        # for companies 0-1, biases, W2) go on the scalar ring.
        zall = const.tile([128, CPC, NTT, KC, TW], dt_op)
        b1t = const.tile([128, CPC * 2 * KC], f32)
        w2t = const.tile([128, CPC * HC], dt_op)

        w1p = ctx.enter_context(tc.tile_pool(name="w1p", bufs=1))
        w1ts = []
        for c in range(CPC):
            w1t = w1p.tile([128, 2, KC, H // 2], dt_op, name=f"w1_{c}")
            nc.sync.dma_start(out=w1t[:], in_=w1_d[c])
            w1ts.append(w1t)
            if c == 1:
                nc.sync.dma_start(out=zall[:, 2:], in_=zt_d[:, 2:])

        nc.scalar.dma_start(out=zall[:, 0:1], in_=zt_d[:, 0:1])
        nc.scalar.dma_start(out=zall[:, 1:2], in_=zt_d[:, 1:2])
        nc.scalar.dma_start(out=b1t[:], in_=b1_d[:])
        nc.scalar.dma_start(out=w2t[:], in_=w2_d[:])

        # Staged per-company outputs; two HWDGE stores at the end.
        oall = const.tile([1, CPC * NTT * TW], f32)

        hp = ctx.enter_context(tc.tile_pool(name="hp", bufs=4))
        pp = ctx.enter_context(tc.tile_pool(name="pp", bufs=4, space="PSUM"))
        opp = ctx.enter_context(tc.tile_pool(name="opp", bufs=2, space="PSUM"))

        # PE warmup: dependency-free matmuls on scratch data so the HAM
        # clock gate un-throttles the PE (1.2 -> 2.4 GHz takes ~3.4us of
        # sustained activity) while the first w1 DMA streams in. The
        # scratch memset runs on the otherwise-idle vector engine so the
        # warmup starts right after the framework preamble.
        wsc = const.tile([128, 384], dt_op)
        nc.vector.memset(wsc[:], 0.0)
        wps = ctx.enter_context(tc.tile_pool(name="wps", bufs=1, space="PSUM"))
        wp = wps.tile([128, 384], f32)
        for _ in range(10):
            nc.tensor.matmul(wp[:], wsc[:, :128], wsc[:], start=True, stop=True)

        # Software-pipelined job list: layer-2 of job i is emitted after
        # layer-1 of job i+1, so the PE keeps streaming layer-1 matmuls
        # while the ACT engine computes job i's gelu.
        jobs = [(c, tt, g) for c in range(CPC) for tt in range(NTT) for g in range(2)]
        pending = []  # (c, tt, g, ps_or_ht state)
        osums = {}

        def emit_l1(c, tt, g):
            ps = pp.tile([128, KC * TW], f32)
            for j in range(KC):
                for k in range(KC):
                    nc.tensor.matmul(
                        ps[:, j * TW:(j + 1) * TW],
                        w1ts[c][:, g, k, 128 * j:128 * (j + 1)],
                        zall[:, c, tt, k, :],
                        start=(k == 0),
                        stop=(k == KC - 1),
                    )
            # Bias on the (otherwise idle) vector engine: ONE in-place
            # broadcast add into PSUM per group. A stride-0 AP repeats
            # each b1 value across the token axis, so the whole group is
            # a single DVE instruction — per-instruction fixed cost
            # (~0.4us) on DVE/ACT is what paced the previous versions.
            idx = (c * 2 + g) * KC
            bias_b = b1t[:, idx:idx + KC].unsqueeze(2).broadcast_to([128, KC, TW])
            psv = ps[:].rearrange("p (j t) -> p j t", j=KC)
            nc.vector.tensor_add(psv, psv, bias_b)
            ht = hp.tile([128, KC * TW], dt_op)
            nc.scalar.activation(ht[:], ps[:], gelu)
            return ht

        def emit_l2(c, tt, g, ht):
            if g == 0:
                osums[(c, tt)] = opp.tile([1, TW], f32, name="osum")
            osum = osums[(c, tt)]
            for j in range(KC):
                jj = KC * g + j
                nc.tensor.matmul(
                    osum[:],
                    w2t[:, HC * c + jj:HC * c + jj + 1],
                    ht[:, j * TW:(j + 1) * TW],
                    start=(jj == 0),
                    stop=(jj == HC - 1),
                )
            if g == 1:
                off = (c * NTT + tt) * TW
                nc.vector.tensor_copy(oall[:, off:off + TW], osum[:])
                del osums[(c, tt)]

        for job in jobs:
            ht = emit_l1(*job)
            if pending:
                emit_l2(*pending.pop(0))
            pending.append((*job, ht))
        while pending:
            emit_l2(*pending.pop(0))

        # Stores on the sync ring (drained long before): companies 0-6
        # fire while company 7 computes; the final store is tiny.
        osplit = (CPC - 1) * NTT * TW
        nc.sync.dma_start(out=out_d[:, :osplit], in_=oall[:, :osplit])
        nc.sync.dma_start(out=out_d[:, osplit:], in_=oall[:, osplit:])

    nc.finalize()
    return nc


def _get_compiled(TW, NTT, dtype_name):
    key = (TW, NTT, dtype_name)
    if key not in _COMPILED:
        _COMPILED[key] = _build(TW, NTT, dtype_name)
    return _COMPILED[key]


def kernel(z, company_id, W1, b1, W2, b2):
    from concourse.bass_utils import run_bass_kernel_spmd

    z = np.asarray(z, dtype=np.float32)
    cid = np.asarray(company_id).astype(np.int64).ravel()
    W1 = np.asarray(W1, dtype=np.float32)
    b1 = np.asarray(b1, dtype=np.float32)
    W2 = np.asarray(W2, dtype=np.float32)
    b2 = np.asarray(b2, dtype=np.float32)
    O = W2.shape[2]

    np_op = np.float16
    dtype_name = "float16"

    idx_by_company = [np.nonzero(cid == gc)[0] for gc in range(C)]
    max_cnt = max((len(ix) for ix in idx_by_company), default=1)
    max_cnt = max(max_cnt, 1)
    if max_cnt <= 128:
        NTT = 1
        TW = ((max_cnt + 15) // 16) * 16
    else:
        NTT = (max_cnt + 127) // 128
        TW = 128
    CAP = NTT * TW

    nc = _get_compiled(TW, NTT, dtype_name)

    in_maps = []
    for core in range(NCORES):
        # zt[p, c, tt, k, t] = z[token, 128k+p]  (partition-major)
        zt = np.zeros((128, CPC, NTT, KC, TW), dtype=np_op)
        for ci in range(CPC):
            gc = core * CPC + ci
            ix = idx_by_company[gc]
            if len(ix) == 0:
                continue
            zpad = np.zeros((CAP, D), dtype=np_op)
            zpad[: len(ix)] = z[ix].astype(np_op)
            zt[:, ci] = zpad.reshape(NTT, TW, KC, 128).transpose(3, 0, 2, 1)
        # w1[c, p, g, k, hh] = W1[gc, 128k+p, 512g+hh]
        w1 = (
            W1[core * CPC:(core + 1) * CPC]
            .reshape(CPC, KC, 128, 2, H // 2)
            .transpose(0, 2, 3, 1, 4)
            .astype(np_op)
        )
        # b1c[m, (c*2+g)*KC + j] = b1[gc, 512g+128j+m]  (fp32 columns)
        b1c = (
            b1[core * CPC:(core + 1) * CPC]
            .reshape(CPC, 2, KC, 128)
            .transpose(3, 0, 1, 2)
            .reshape(128, CPC * 2 * KC)
            .astype(np.float32)
        )
        # w2h[p, HC*c + j] = W2[gc, 128j+p, 0]
        w2h = (
            W2[core * CPC:(core + 1) * CPC, :, 0]
            .reshape(CPC, HC, 128)
            .transpose(2, 0, 1)
            .reshape(128, CPC * HC)
            .astype(np_op)
        )
        in_maps.append(
            {
                "zt": np.ascontiguousarray(zt),
                "w1": np.ascontiguousarray(w1),
                "b1c": np.ascontiguousarray(b1c),
                "w2h": np.ascontiguousarray(w2h),
            }
        )

    res = run_bass_kernel_spmd(nc, in_maps, list(range(NCORES)))

    out = np.zeros((B, O), dtype=np.float32)
    for core in range(NCORES):
        core_out = res.results[core]["out"].reshape(CPC, NTT * TW)
        for ci in range(CPC):
            gc = core * CPC + ci
            ix = idx_by_company[gc]
            if len(ix) == 0:
                continue
            out[ix, 0] = core_out[ci, : len(ix)] + b2[gc, 0]
    return out


# revision 14
# speedup vs baseline: 1.1048x; 1.0626x over previous
"""Trainium2 Bass kernel for CompanySpecificHeads (MoE-style routed MLP heads).

Semantics (matching the reference):
    out[b] = gelu(z[b] @ W1[cid[b]] + b1[cid[b]]) @ W2[cid[b]] + b2[cid[b]]

Strategy: expert-parallel across 8 NeuronCores. Companies are sharded
8-per-core; tokens are routed (gathered by company) to their company's core
on the host, padded to a fixed per-company capacity, and each core runs a
grouped GEMM -> gelu -> dot pipeline over its 8 companies:

  Layer 1 (per company c, h on partitions):
      psum[h, t] = sum_d W1[c][d, h] * zT[c][d, t]      (PE, fp16 operands)
      bias b1 is folded in with a K=4 "selector" matmul that broadcasts
      b1[c][128k+m] across the token axis before accumulation.
  Gelu: ACT engine, PSUM -> SBUF (fp16 out).
  Layer 2: psum2[1, t] += W2[c][hj]^T @ gelu_h[hj, t]   (8 K=128 matmuls)

Host does the unshard/scatter back to [B, 1] and adds b2 (exact, fp32).

DMA discipline: the DIRECT2D DMA encoding supports a single sync wait, so
the kernel keeps every DMACopy at <=1 wait: all loads target fresh SBUF
slots (no reuse -> no release waits), there are <=8 DMAs per DGE flavor
(fresh lane -> no FIFO wait), and the 8 per-company outputs are staged into
one persistent SBUF tile and stored with two sync-ring DMAs at the end.

PE warmup: the HAM clock gate holds an idle PE at 1.2GHz and takes ~3.4us
of sustained activity to un-throttle to 2.4GHz. Warmup matmuls on scratch
data bridge the gap while w1[0] streams in; their scratch memset runs on
the otherwise-idle vector engine so they start right after the framework
preamble (~7us) instead of being queued behind gpsimd DMA dispatches.
"""

import numpy as np

B, C, D, H = 4096, 64, 512, 1024
NCORES = 8
CPC = C // NCORES  # companies per core
KC = D // 128      # contraction chunks of 128
HC = H // 128      # h chunks of 128

_COMPILED = {}


def _build(TW, NTT, dtype_name):
    """Build the Bass/Tile program for per-company token capacity NTT*TW."""
    import concourse.bass as bass
    import concourse.bacc as bacc
    import concourse.mybir as mybir
    from concourse.tile import TileContext
    from contextlib import ExitStack

    f32 = mybir.dt.float32
    dt_op = getattr(mybir.dt, dtype_name)

    SELW = KC * TW           # selector columns
    B1W = CPC * 2 * 128      # b1 columns

    nc = bacc.Bacc(None, target_bir_lowering=False)

    # zt is stored partition-major so one DMA moves it with large packets.
    zt_d = nc.dram_tensor("zt", [128, CPC, NTT, KC, TW], dt_op, kind="ExternalInput")
    # w1 stored as [c][p][g][k][h-half]: a whole company loads linearly
    # with 8KB contiguous per partition (full-rate packets).
    w1_d = nc.dram_tensor(
        "w1", [CPC, 128, 2, KC, H // 2], dt_op, kind="ExternalInput"
    )
    cst_d = nc.dram_tensor("cst", [KC, SELW + B1W], dt_op, kind="ExternalInput")
    w2_d = nc.dram_tensor("w2h", [128, CPC * HC], dt_op, kind="ExternalInput")
    out_d = nc.dram_tensor("out", [1, CPC * NTT * TW], f32, kind="ExternalOutput")

    gelu = mybir.ActivationFunctionType.Gelu

    with TileContext(nc) as tc, ExitStack() as ctx:
        const = ctx.enter_context(tc.tile_pool(name="const", bufs=1))

        # PE warmup scratch: memset on the otherwise-idle vector engine so
        # the warmup matmuls have no dependency on the DMA-dispatch queues.
        wsc = const.tile([128, 384], dt_op)
        nc.vector.memset(wsc[:], 0.0)

        # Small constants: [sel | b1h] (4 partitions) and w2 (128 partitions).
        # On the gpsimd SWDGE ring so they land before w1[0] and the PE can
        # run the bias matmuls while weights stream in.
        ct = const.tile([KC, SELW + B1W], dt_op)
        nc.gpsimd.dma_start(out=ct[:], in_=cst_d[:])
        selt = ct[:, 0:SELW]
        b1t = ct[:, SELW:SELW + B1W].rearrange("p (c g m) -> p c g m", c=CPC, g=2)
        w2t = const.tile([128, CPC * HC], dt_op)
        nc.gpsimd.dma_start(out=w2t[:], in_=w2_d[:])

        # Routed tokens on the ACT HWDGE ring (its dispatch overlaps the SP
        # ring's w1 dispatches): first company lands early so the PE can
        # start as soon as w1[0] arrives.
        zall = const.tile([128, CPC, NTT, KC, TW], dt_op)
        zsplit = 1
        nc.scalar.dma_start(out=zall[:, :zsplit], in_=zt_d[:, :zsplit])
        if zsplit < CPC:
            nc.scalar.dma_start(out=zall[:, zsplit:], in_=zt_d[:, zsplit:])

        # Staged per-company outputs; two sync-ring stores at the end.
        oall = const.tile([1, CPC * NTT * TW], f32)

        # Per-company weights on the SP HWDGE ring, one company per DMA.
        # The ring drains FIFO at full bandwidth, so w1 chunks complete in
        # issue order and compute pipelines behind the weight stream.
        w1p = ctx.enter_context(tc.tile_pool(name="w1p", bufs=1))
        w1ts = []
        for c in range(CPC):
            w1t = w1p.tile([128, 2, KC, H // 2], dt_op, name=f"w1_{c}")
            nc.sync.dma_start(out=w1t[:], in_=w1_d[c])
            w1ts.append(w1t)

        hp = ctx.enter_context(tc.tile_pool(name="hp", bufs=min(2 * CPC * NTT, 16)))
        pp = ctx.enter_context(tc.tile_pool(name="pp", bufs=5, space="PSUM"))
        opp = ctx.enter_context(tc.tile_pool(name="opp", bufs=2, space="PSUM"))

        # PE warmup matmuls: ~12 x 384-free at the cold 1.2GHz clock is
        # ~3.8us of activity -- the HAM un-throttles right around when
        # w1[0] lands, and the queue drains just in time for real work.
        wps = ctx.enter_context(tc.tile_pool(name="wps", bufs=1, space="PSUM"))
        wp = wps.tile([128, 384], f32)
        for _ in range(12):
            nc.tensor.matmul(wp[:], wsc[:, :128], wsc[:], start=True, stop=True)

        for c in range(CPC):
            w1t = w1ts[c]
            for tt in range(NTT):
                osum = opp.tile([1, TW], f32)
                for g in range(2):
                    ps = pp.tile([128, KC * TW], f32)
                    # bias: ps[128k+m, (j,t)] = b1[c][512g+128j+m] via selector
                    nc.tensor.matmul(
                        ps[:], b1t[:, c, g, :], selt[:], start=True, stop=False
                    )
                    for j in range(KC):
                        for k in range(KC):
                            nc.tensor.matmul(
                                ps[:, j * TW:(j + 1) * TW],
                                w1t[:, g, k, 128 * j:128 * (j + 1)],
                                zall[:, c, tt, k, :],
                                start=False,
                                stop=(k == KC - 1),
                            )
                    ht = hp.tile([128, KC * TW], dt_op)
                    nc.scalar.activation(ht[:], ps[:], gelu)
                    for j in range(KC):
                        jj = KC * g + j
                        nc.tensor.matmul(
                            osum[:],
                            w2t[:, HC * c + jj:HC * c + jj + 1],
                            ht[:, j * TW:(j + 1) * TW],
                            start=(jj == 0),
                            stop=(jj == HC - 1),
                        )
                off = (c * NTT + tt) * TW
                nc.vector.tensor_copy(oall[:, off:off + TW], osum[:])

        # Stores on the sync ring (HWDGE, sub-us completion): companies
        # 0-6 fire while company 7 computes; the final store is tiny.
        osplit = (CPC - 1) * NTT * TW
        nc.sync.dma_start(out=out_d[:, :osplit], in_=oall[:, :osplit])
        nc.sync.dma_start(out=out_d[:, osplit:], in_=oall[:, osplit:])

    nc.finalize()
    return nc


def _get_compiled(TW, NTT, dtype_name):
    key = (TW, NTT, dtype_name)
    if key not in _COMPILED:
        _COMPILED[key] = _build(TW, NTT, dtype_name)
    return _COMPILED[key]


def kernel(z, company_id, W1, b1, W2, b2):
    from concourse.bass_utils import run_bass_kernel_spmd

    z = np.asarray(z, dtype=np.float32)
    cid = np.asarray(company_id).astype(np.int64).ravel()
    W1 = np.asarray(W1, dtype=np.float32)
    b1 = np.asarray(b1, dtype=np.float32)
    W2 = np.asarray(W2, dtype=np.float32)
    b2 = np.asarray(b2, dtype=np.float32)
    O = W2.shape[2]

    np_op = np.float16
    dtype_name = "float16"

    idx_by_company = [np.nonzero(cid == gc)[0] for gc in range(C)]
    max_cnt = max((len(ix) for ix in idx_by_company), default=1)
    max_cnt = max(max_cnt, 1)
    if max_cnt <= 128:
        NTT = 1
        TW = ((max_cnt + 15) // 16) * 16
    else:
        NTT = (max_cnt + 127) // 128
        TW = 128
    CAP = NTT * TW

    nc = _get_compiled(TW, NTT, dtype_name)

    SELW = KC * TW
    B1W = CPC * 2 * 128
    sel = np.repeat(np.eye(KC, dtype=np_op), TW, axis=1)  # [KC, KC*TW]

    in_maps = []
    for core in range(NCORES):
        # zt[p, c, tt, k, t] = z[token, 128k+p]  (partition-major)
        zt = np.zeros((128, CPC, NTT, KC, TW), dtype=np_op)
        for ci in range(CPC):
            gc = core * CPC + ci
            ix = idx_by_company[gc]
            if len(ix) == 0:
                continue
            zpad = np.zeros((CAP, D), dtype=np_op)
            zpad[: len(ix)] = z[ix].astype(np_op)
            zt[:, ci] = zpad.reshape(NTT, TW, KC, 128).transpose(3, 0, 2, 1)
        # w1[c, p, g, k, hh] = W1[gc, 128k+p, 512g+hh]
        w1 = (
            W1[core * CPC:(core + 1) * CPC]
            .reshape(CPC, KC, 128, 2, H // 2)
            .transpose(0, 2, 3, 1, 4)
            .astype(np_op)
        )
        # b1h[k, c, g, m] = b1[gc, 512g+128k+m]
        b1h = (
            b1[core * CPC:(core + 1) * CPC]
            .reshape(CPC, 2, KC, 128)
            .transpose(2, 0, 1, 3)
            .astype(np_op)
        )
        # w2h[p, HC*c + j] = W2[gc, 128j+p, 0]
        w2h = (
            W2[core * CPC:(core + 1) * CPC, :, 0]
            .reshape(CPC, HC, 128)
            .transpose(2, 0, 1)
            .reshape(128, CPC * HC)
            .astype(np_op)
        )
        cst = np.zeros((KC, SELW + B1W), dtype=np_op)
        cst[:, 0:SELW] = sel
        cst[:, SELW:SELW + B1W] = b1h.reshape(KC, B1W)
        in_maps.append(
            {
                "zt": np.ascontiguousarray(zt),
                "w1": np.ascontiguousarray(w1),
                "cst": np.ascontiguousarray(cst),
                "w2h": np.ascontiguousarray(w2h),
            }
        )

    res = run_bass_kernel_spmd(nc, in_maps, list(range(NCORES)))

    out = np.zeros((B, O), dtype=np.float32)
    for core in range(NCORES):
        core_out = res.results[core]["out"].reshape(CPC, NTT * TW)
        for ci in range(CPC):
            gc = core * CPC + ci
            ix = idx_by_company[gc]
            if len(ix) == 0:
                continue
            out[ix, 0] = core_out[ci, : len(ix)] + b2[gc, 0]
    return out


# revision 15
# speedup vs baseline: 1.2348x; 1.1177x over previous
"""Trainium2 Bass kernel for CompanySpecificHeads (MoE-style routed MLP heads).

Semantics (matching the reference):
    out[b] = gelu(z[b] @ W1[cid[b]] + b1[cid[b]]) @ W2[cid[b]] + b2[cid[b]]

Strategy: expert-parallel across 8 NeuronCores. Companies are sharded
8-per-core; tokens are routed (gathered by company) to their company's core
on the host, padded to a fixed per-company capacity, and each core runs a
grouped GEMM -> gelu -> dot pipeline over its 8 companies:

  Layer 1 (per company c, h on partitions):
      psum[h, t] = sum_d W1[c][d, h] * zT[c][d, t]      (PE, fp16 operands)
      bias b1 is folded in with a K=4 "selector" matmul that broadcasts
      b1[c][128k+m] across the token axis before accumulation.
  Gelu: ACT engine, PSUM -> SBUF (fp16 out).
  Layer 2: psum2[1, t] += W2[c][hj]^T @ gelu_h[hj, t]   (8 K=128 matmuls)

Host does the unshard/scatter back to [B, 1] and adds b2 (exact, fp32).

DMA discipline: the DIRECT2D DMA encoding supports a single sync wait, so
the kernel keeps every DMACopy at <=1 wait: all loads target fresh SBUF
slots (no reuse -> no release waits), there are <=8 DMAs per DGE flavor
(fresh lane -> no FIFO wait), and the 8 per-company outputs are staged into
one persistent SBUF tile and stored with two sync-ring DMAs at the end.

PE warmup: the HAM clock gate holds an idle PE at 1.2GHz and takes ~3.4us
of sustained activity to un-throttle to 2.4GHz. Warmup matmuls on scratch
data bridge the gap while w1[0] streams in; their scratch memset runs on
the otherwise-idle vector engine so they start right after the framework
preamble (~7us) instead of being queued behind gpsimd DMA dispatches.
"""

import numpy as np

B, C, D, H = 4096, 64, 512, 1024
NCORES = 8
CPC = C // NCORES  # companies per core
KC = D // 128      # contraction chunks of 128
HC = H // 128      # h chunks of 128

_COMPILED = {}


def _build(TW, NTT, dtype_name):
    """Build the Bass/Tile program for per-company token capacity NTT*TW."""
    import concourse.bass as bass
    import concourse.bacc as bacc
    import concourse.mybir as mybir
    from concourse.tile import TileContext
    from contextlib import ExitStack

    f32 = mybir.dt.float32
    dt_op = getattr(mybir.dt, dtype_name)

    SELW = KC * TW           # selector columns
    B1W = CPC * 2 * 128      # b1 columns

    nc = bacc.Bacc(None, target_bir_lowering=False)

    # zt is stored partition-major so one DMA moves it with large packets.
    zt_d = nc.dram_tensor("zt", [128, CPC, NTT, KC, TW], dt_op, kind="ExternalInput")
    # w1 stored as [c][p][g][k][h-half]: a whole company loads linearly
    # with 8KB contiguous per partition (full-rate packets).
    w1_d = nc.dram_tensor(
        "w1", [CPC, 128, 2, KC, H // 2], dt_op, kind="ExternalInput"
    )
    cst_d = nc.dram_tensor("cst", [KC, SELW + B1W], dt_op, kind="ExternalInput")
    w2_d = nc.dram_tensor("w2h", [128, CPC * HC], dt_op, kind="ExternalInput")
    out_d = nc.dram_tensor("out", [1, CPC * NTT * TW], f32, kind="ExternalOutput")

    gelu = mybir.ActivationFunctionType.Gelu

    with TileContext(nc) as tc, ExitStack() as ctx:
        const = ctx.enter_context(tc.tile_pool(name="const", bufs=1))

        # PE warmup scratch: memset on the otherwise-idle vector engine so
        # the warmup matmuls have no dependency on the DMA-dispatch queues.
        wsc = const.tile([128, 384], dt_op)
        nc.vector.memset(wsc[:], 0.0)

        # Everything that gates the pipeline rides the sync (SP HWDGE)
        # ring in need-order -- it is FIFO and the only ring that
        # sustains full rate, so arrivals are deterministic:
        #   cst (bias/selector, gates company 0's first matmul),
        #   zt[0] (company 0 tokens), w1[0], zt[1:] (needed from company
        #   1 at ~16us), then w1[1..7] paced by the stream itself.
        # The scalar ring only crawls (~110GB/s) when competing with the
        # sync stream, and gpsimd SWDGE delivers late (~13-15us) -- both
        # starved the pipeline head in earlier versions and re-throttled
        # the PE clock mid-kernel. Only w2 (needed at ~15us) stays on
        # gpsimd.
        ct = const.tile([KC, SELW + B1W], dt_op)
        nc.sync.dma_start(out=ct[:], in_=cst_d[:])
        selt = ct[:, 0:SELW]
        b1t = ct[:, SELW:SELW + B1W].rearrange("p (c g m) -> p c g m", c=CPC, g=2)

        zall = const.tile([128, CPC, NTT, KC, TW], dt_op)
        nc.sync.dma_start(out=zall[:, 0:1], in_=zt_d[:, 0:1])

        w2t = const.tile([128, CPC * HC], dt_op)
        nc.gpsimd.dma_start(out=w2t[:], in_=w2_d[:])

        # Staged per-company outputs; two sync-ring stores at the end.
        oall = const.tile([1, CPC * NTT * TW], f32)

        # Per-company weights, one company per DMA; tokens for companies
        # 1-7 slot in right after w1[0].
        w1p = ctx.enter_context(tc.tile_pool(name="w1p", bufs=1))
        w1ts = []
        for c in range(CPC):
            w1t = w1p.tile([128, 2, KC, H // 2], dt_op, name=f"w1_{c}")
            nc.sync.dma_start(out=w1t[:], in_=w1_d[c])
            w1ts.append(w1t)
            if c == 0:
                nc.sync.dma_start(out=zall[:, 1:], in_=zt_d[:, 1:])

        hp = ctx.enter_context(tc.tile_pool(name="hp", bufs=min(2 * CPC * NTT, 16)))
        pp = ctx.enter_context(tc.tile_pool(name="pp", bufs=5, space="PSUM"))
        opp = ctx.enter_context(tc.tile_pool(name="opp", bufs=2, space="PSUM"))

        # PE warmup matmuls: ~12 x 384-free at the cold 1.2GHz clock is
        # ~3.8us of activity -- the HAM un-throttles right around when
        # w1[0] lands, and the queue drains just in time for real work.
        wps = ctx.enter_context(tc.tile_pool(name="wps", bufs=1, space="PSUM"))
        wp = wps.tile([128, 384], f32)
        for _ in range(12):
            nc.tensor.matmul(wp[:], wsc[:, :128], wsc[:], start=True, stop=True)

        for c in range(CPC):
            w1t = w1ts[c]
            for tt in range(NTT):
                osum = opp.tile([1, TW], f32)
                for g in range(2):
                    ps = pp.tile([128, KC * TW], f32)
                    # bias: ps[128k+m, (j,t)] = b1[c][512g+128j+m] via selector
                    nc.tensor.matmul(
                        ps[:], b1t[:, c, g, :], selt[:], start=True, stop=False
                    )
                    for j in range(KC):
                        for k in range(KC):
                            nc.tensor.matmul(
                                ps[:, j * TW:(j + 1) * TW],
                                w1t[:, g, k, 128 * j:128 * (j + 1)],
                                zall[:, c, tt, k, :],
                                start=False,
                                stop=(k == KC - 1),
                            )
                    ht = hp.tile([128, KC * TW], dt_op)
                    nc.scalar.activation(ht[:], ps[:], gelu)
                    for j in range(KC):
                        jj = KC * g + j
                        nc.tensor.matmul(
                            osum[:],
                            w2t[:, HC * c + jj:HC * c + jj + 1],
                            ht[:, j * TW:(j + 1) * TW],
                            start=(jj == 0),
                            stop=(jj == HC - 1),
                        )
                off = (c * NTT + tt) * TW
                nc.vector.tensor_copy(oall[:, off:off + TW], osum[:])

        # Stores on the sync ring (HWDGE, sub-us completion): companies
        # 0-6 fire while company 7 computes; the final store is tiny.
        osplit = (CPC - 1) * NTT * TW
        nc.sync.dma_start(out=out_d[:, :osplit], in_=oall[:, :osplit])
        nc.sync.dma_start(out=out_d[:, osplit:], in_=oall[:, osplit:])

    nc.finalize()
    return nc


def _get_compiled(TW, NTT, dtype_name):
    key = (TW, NTT, dtype_name)
    if key not in _COMPILED:
        _COMPILED[key] = _build(TW, NTT, dtype_name)
    return _COMPILED[key]


def kernel(z, company_id, W1, b1, W2, b2):
    from concourse.bass_utils import run_bass_kernel_spmd

    z = np.asarray(z, dtype=np.float32)
    cid = np.asarray(company_id).astype(np.int64).ravel()
    W1 = np.asarray(W1, dtype=np.float32)
    b1 = np.asarray(b1, dtype=np.float32)
    W2 = np.asarray(W2, dtype=np.float32)
    b2 = np.asarray(b2, dtype=np.float32)
    O = W2.shape[2]

    np_op = np.float16
    dtype_name = "float16"

    idx_by_company = [np.nonzero(cid == gc)[0] for gc in range(C)]
    max_cnt = max((len(ix) for ix in idx_by_company), default=1)
    max_cnt = max(max_cnt, 1)
    if max_cnt <= 128:
        NTT = 1
        TW = ((max_cnt + 15) // 16) * 16
    else:
        NTT = (max_cnt + 127) // 128
        TW = 128
    CAP = NTT * TW

    nc = _get_compiled(TW, NTT, dtype_name)

    SELW = KC * TW
    B1W = CPC * 2 * 128
    sel = np.repeat(np.eye(KC, dtype=np_op), TW, axis=1)  # [KC, KC*TW]

    in_maps = []
    for core in range(NCORES):
        # zt[p, c, tt, k, t] = z[token, 128k+p]  (partition-major)
        zt = np.zeros((128, CPC, NTT, KC, TW), dtype=np_op)
        for ci in range(CPC):
            gc = core * CPC + ci
            ix = idx_by_company[gc]
            if len(ix) == 0:
                continue
            zpad = np.zeros((CAP, D), dtype=np_op)
            zpad[: len(ix)] = z[ix].astype(np_op)
            zt[:, ci] = zpad.reshape(NTT, TW, KC, 128).transpose(3, 0, 2, 1)
        # w1[c, p, g, k, hh] = W1[gc, 128k+p, 512g+hh]
        w1 = (
            W1[core * CPC:(core + 1) * CPC]
            .reshape(CPC, KC, 128, 2, H // 2)
            .transpose(0, 2, 3, 1, 4)
            .astype(np_op)
        )
        # b1h[k, c, g, m] = b1[gc, 512g+128k+m]
        b1h = (
            b1[core * CPC:(core + 1) * CPC]
            .reshape(CPC, 2, KC, 128)
            .transpose(2, 0, 1, 3)
            .astype(np_op)
        )
        # w2h[p, HC*c + j] = W2[gc, 128j+p, 0]
        w2h = (
            W2[core * CPC:(core + 1) * CPC, :, 0]
            .reshape(CPC, HC, 128)
            .transpose(2, 0, 1)
            .reshape(128, CPC * HC)
            .astype(np_op)
        )
        cst = np.zeros((KC, SELW + B1W), dtype=np_op)
        cst[:, 0:SELW] = sel
        cst[:, SELW:SELW + B1W] = b1h.reshape(KC, B1W)
        in_maps.append(
            {
                "zt": np.ascontiguousarray(zt),
                "w1": np.ascontiguousarray(w1),
                "cst": np.ascontiguousarray(cst),
                "w2h": np.ascontiguousarray(w2h),
            }
        )

    res = run_bass_kernel_spmd(nc, in_maps, list(range(NCORES)))

    out = np.zeros((B, O), dtype=np.float32)
    for core in range(NCORES):
        core_out = res.results[core]["out"].reshape(CPC, NTT * TW)
        for ci in range(CPC):
            gc = core * CPC + ci
            ix = idx_by_company[gc]
            if len(ix) == 0:
                continue
            out[ix, 0] = core_out[ci, : len(ix)] + b2[gc, 0]
    return out
